# revision 5
# baseline (speedup 1.0000x reference)
"""Trainium2 Bass kernel: MechanicsPINN residual (MLP field + biharmonic stencil).

Math (reference): f = MLP(x_coloc) -> [B, H*W]; residual = L(L(f)) + L(f) + f - P
where L is the 5-point reflect-padded Laplacian (EI = KC = GC = 1, dx = dy = 1).

Sharding: tensor-parallel over the 65536 output pixels = 256 image rows.
Core c owns rows [32c, 32c+32). Each core recomputes the tiny MLP, then
computes f for its rows plus a 2-row halo on each side (mirror boundary rows
are folded in on the host by remapping which W4 columns each core streams,
so the device stencil needs no y-boundary cases and no cross-core comms).

Device layout: batch (64) on partitions; each core's 32 rows are split into
two 16-row halves stacked on the partition axis (partitions 0-63 = batch for
half A, 64-127 = batch for half B) via PE column-tiling, so DVE stencil
passes and the big matmul both use all 128 partitions. W4 is streamed in
bf16 (host-cast); all stencil math is fp32.
"""

import numpy as np
import ml_dtypes

import concourse.bass as bass
import concourse.tile as tile
from concourse import bacc, mybir
from concourse.bass_utils import run_bass_kernel_spmd

F32 = mybir.dt.float32
BF16 = mybir.dt.bfloat16
BF16_NP = ml_dtypes.bfloat16

B = 64          # batch (collocation samples)
H = 256         # image rows
W = 256         # image cols
NCORES = 8
OWN = 32        # image rows owned per core
HR = 16         # rows per half-slab
FR = 20         # f rows held per half (HR + 2 halo each side)
LR = 18         # laplacian rows per half (HR + 1 each side)
PAIRS = 10      # chunk pairs: FR*W / 512 columns
KT = 8          # k tiles of the 1024-dim contraction

_PROGRAM_CACHE = {}


def _mirror(j):
    # jnp.pad mode='reflect' (no edge repeat): p[-1] = f[1], p[H] = f[H-2]
    if j < 0:
        return -j
    if j > H - 1:
        return 2 * (H - 1) - j
    return j


def _build_program():
    nc = bacc.Bacc("TRN2", target_bir_lowering=False, debug=False)

    xT = nc.declare_dram_parameter("xT", [2, B], F32, isOutput=False)
    W1 = nc.declare_dram_parameter("W1", [2, 256], F32, isOutput=False)
    b1 = nc.declare_dram_parameter("b1", [2, 128, 1], F32, isOutput=False)
    W2 = nc.declare_dram_parameter("W2", [2, 128, 512], F32, isOutput=False)
    b2 = nc.declare_dram_parameter("b2", [4, 128, 1], F32, isOutput=False)
    W3 = nc.declare_dram_parameter("W3", [4, 128, 1024], F32, isOutput=False)
    b3 = nc.declare_dram_parameter("b3", [8, 128, 1], F32, isOutput=False)
    W4s = nc.declare_dram_parameter("W4s", [PAIRS, 128, KT, 1024], BF16, isOutput=False)
    b4s = nc.declare_dram_parameter("b4s", [PAIRS, 1, 1024], BF16, isOutput=False)
    Ps = nc.declare_dram_parameter("Ps", [128, HR * W], F32, isOutput=False)
    out = nc.declare_dram_parameter("out", [128, HR * W], F32, isOutput=True)

    Relu = mybir.ActivationFunctionType.Relu
    MUL = mybir.AluOpType.mult
    ADD = mybir.AluOpType.add

    with tile.TileContext(nc) as tc:
        with (
            tc.tile_pool(name="singles", bufs=1) as singles,
            tc.tile_pool(name="wpool", bufs=2) as wpool,
            tc.tile_pool(name="bpool", bufs=2) as bpool,
            tc.tile_pool(name="spool", bufs=2) as spool,
            tc.tile_pool(name="tpool", bufs=2) as tpool,
            tc.tile_pool(name="rpool", bufs=2) as rpool,
        ):
            dma = nc.sync.dma_start

            xT_sb = singles.tile([2, B], F32)
            W1_sb = singles.tile([2, 256], F32)
            b1_sb = singles.tile([128, 2], F32)
            W2_sb = singles.tile([128, 2, 512], F32)
            b2_sb = singles.tile([128, 4], F32)
            W3_sb = singles.tile([128, 4, 1024], F32)
            b3_sb = singles.tile([128, 8], F32)
            h1_sb = singles.tile([128, 2, B], F32)
            h2_sb = singles.tile([128, 4, B], F32)
            h3_sb = singles.tile([128, KT, B], BF16)
            ones = singles.tile([1, B], BF16)
            Ft = singles.tile([128, FR * W], F32)
            Lf = singles.tile([128, LR * W], F32)
            Ps_sb = singles.tile([128, HR * W], F32)

            dma(out=xT_sb[:, :], in_=xT[:, :])
            dma(out=W1_sb[:, :], in_=W1[:, :])
            for k in range(2):
                dma(out=b1_sb[:, k : k + 1], in_=b1[k])
                dma(out=W2_sb[:, k, :], in_=W2[k])
            for k in range(4):
                dma(out=b2_sb[:, k : k + 1], in_=b2[k])
                dma(out=W3_sb[:, k, :], in_=W3[k])
            for k in range(8):
                dma(out=b3_sb[:, k : k + 1], in_=b3[k])
            dma(out=Ps_sb[:, :], in_=Ps[:, :])
            nc.vector.memset(ones, 1.0)

            # ---- MLP (transposed activations: h_T[feat, batch]) ----
            with tc.tile_pool(name="mlp_psum", bufs=2, space="PSUM") as mp:
                for m in range(2):
                    ps = mp.tile([128, B], F32)
                    nc.tensor.matmul(
                        ps, W1_sb[:, m * 128 : (m + 1) * 128], xT_sb[:, :],
                        start=True, stop=True,
                    )
                    nc.scalar.activation(
                        h1_sb[:, m, :], ps, Relu, bias=b1_sb[:, m : m + 1], scale=1.0
                    )
                for m in range(4):
                    ps = mp.tile([128, B], F32)
                    for k in range(2):
                        nc.tensor.matmul(
                            ps, W2_sb[:, k, m * 128 : (m + 1) * 128], h1_sb[:, k, :],
                            start=(k == 0), stop=(k == 1),
                        )
                    nc.scalar.activation(
                        h2_sb[:, m, :], ps, Relu, bias=b2_sb[:, m : m + 1], scale=1.0
                    )
                for m in range(8):
                    ps = mp.tile([128, B], F32)
                    for k in range(4):
                        nc.tensor.matmul(
                            ps, W3_sb[:, k, m * 128 : (m + 1) * 128], h2_sb[:, k, :],
                            start=(k == 0), stop=(k == 3),
                        )
                    nc.scalar.activation(
                        h3_sb[:, m, :], ps, Relu, bias=b3_sb[:, m : m + 1], scale=1.0
                    )

            # ---- main matmul: F[p, 512-col chunks], half A -> partitions 0-63,
            # half B -> partitions 64-127 (PE column groups run concurrently) ----
            with tc.tile_pool(name="ppool", bufs=3, space="PSUM") as ppool:
                for a in range(PAIRS):
                    wt = wpool.tile([128, KT, 1024], BF16)
                    dma(out=wt[:, :, :], in_=W4s[a])
                    b4t = bpool.tile([1, 1024], BF16)
                    dma(out=b4t[:, :], in_=b4s[a])
                    # half A accumulates in bank 0 (partitions 0-63), half B in
                    # bank 1 (partitions 64-127): separate psum zero regions,
                    # concurrent PE column groups.
                    ps = ppool.tile([128, 1024], F32)
                    for k in range(KT):
                        nc.tensor.matmul(
                            ps[0:64, 0:512], h3_sb[:, k, :], wt[:, k, 0:512],
                            start=(k == 0), stop=False, tile_position=(0, 0),
                        )
                        nc.tensor.matmul(
                            ps[64:128, 512:1024], h3_sb[:, k, :], wt[:, k, 512:1024],
                            start=(k == 0), stop=False, tile_position=(0, 64),
                        )
                    nc.tensor.matmul(
                        ps[0:64, 0:512], ones[:, :], b4t[:, 0:512],
                        start=False, stop=True, tile_position=(0, 0),
                    )
                    nc.tensor.matmul(
                        ps[64:128, 512:1024], ones[:, :], b4t[:, 512:1024],
                        start=False, stop=True, tile_position=(0, 64),
                    )
                    nc.scalar.copy(Ft[0:64, a * 512 : (a + 1) * 512], ps[0:64, 0:512])
                    nc.scalar.copy(Ft[64:128, a * 512 : (a + 1) * 512], ps[64:128, 512:1024])

                # ---- Lf = Laplacian(f): rows 0..17 per half (center = F row+1) ----
                Fv = Ft.rearrange("p (r x) -> p r x", x=W)
                Lfv = Lf.rearrange("p (r x) -> p r x", x=W)
                STT = nc.vector.scalar_tensor_tensor
                for r0 in (0, 6, 12):
                    n = 6 * W
                    cb = (r0 + 1) * W
                    s1 = spool.tile([128, n], F32, tag="s1")
                    s2 = spool.tile([128, n], F32, tag="s2")
                    nc.vector.tensor_add(s1, Ft[:, cb - 1 : cb - 1 + n], Ft[:, cb + 1 : cb + 1 + n])
                    s1v = s1.rearrange("p (r x) -> p r x", x=W)
                    nc.scalar.mul(s1v[:, :, 0:1], Fv[:, r0 + 1 : r0 + 7, 1:2], 2.0)
                    nc.scalar.mul(s1v[:, :, W - 1 : W], Fv[:, r0 + 1 : r0 + 7, W - 2 : W - 1], 2.0)
                    nc.vector.tensor_add(s2, Ft[:, cb - W : cb - W + n], Ft[:, cb + W : cb + W + n])
                    STT(out=s1, in0=Ft[:, cb : cb + n], scalar=-4.0, in1=s1, op0=MUL, op1=ADD)
                    nc.vector.tensor_add(Lf[:, r0 * W : r0 * W + n], s1, s2)

                # ---- residual = L(Lf) + Lf + f - P  (centers: Lf row+1, F row+2) ----
                for r0 in (0, 8):
                    n = 8 * W
                    lb = (r0 + 1) * W
                    fb = (r0 + 2) * W
                    ob = r0 * W
                    t1 = tpool.tile([128, n], F32, tag="t1")
                    t2 = tpool.tile([128, n], F32, tag="t2")
                    rt = rpool.tile([128, n], F32, tag="rt")
                    nc.vector.tensor_add(t1, Lf[:, lb - 1 : lb - 1 + n], Lf[:, lb + 1 : lb + 1 + n])
                    t1v = t1.rearrange("p (r x) -> p r x", x=W)
                    nc.scalar.mul(t1v[:, :, 0:1], Lfv[:, r0 + 1 : r0 + 9, 1:2], 2.0)
                    nc.scalar.mul(t1v[:, :, W - 1 : W], Lfv[:, r0 + 1 : r0 + 9, W - 2 : W - 1], 2.0)
                    nc.vector.tensor_add(t2, Lf[:, lb - W : lb - W + n], Lf[:, lb + W : lb + W + n])
                    STT(out=t1, in0=Lf[:, lb : lb + n], scalar=-3.0, in1=t1, op0=MUL, op1=ADD)
                    nc.vector.tensor_add(t1, t1, t2)
                    nc.vector.tensor_sub(t2, Ft[:, fb : fb + n], Ps_sb[:, ob : ob + n])
                    nc.vector.tensor_add(rt, t1, t2)
                    dma(out=out[:, ob : ob + n], in_=rt[:, :])

    nc.compile()
    return nc


def _ext_rows(c):
    """40 mirrored global row indices: 20 for half A, 20 for half B."""
    y0 = c * OWN
    rows_a = [_mirror(y0 - 2 + j) for j in range(FR)]
    rows_b = [_mirror(y0 + HR - 2 + j) for j in range(FR)]
    return rows_a + rows_b


def _prep_shared(inputs):
    f32 = np.float32
    shared = {
        "xT": np.ascontiguousarray(inputs["x_coloc"].T, dtype=f32),
        "W1": np.ascontiguousarray(inputs["W1"], dtype=f32),
        "b1": np.ascontiguousarray(inputs["b1"], dtype=f32).reshape(2, 128, 1),
        "W2": np.ascontiguousarray(inputs["W2"], dtype=f32).reshape(2, 128, 512),
        "b2": np.ascontiguousarray(inputs["b2"], dtype=f32).reshape(4, 128, 1),
        "W3": np.ascontiguousarray(inputs["W3"], dtype=f32).reshape(4, 128, 1024),
        "b3": np.ascontiguousarray(inputs["b3"], dtype=f32).reshape(8, 128, 1),
    }
    return shared


def _prep_core(c, W4, b4, P):
    rows = _ext_rows(c)
    W4r = W4.reshape(1024, H, W)
    G = W4r[:, rows, :].reshape(KT, 128, 2 * FR, W)       # [k, p, rr, x]
    A = G[:, :, :FR, :].reshape(KT, 128, PAIRS, 512)
    Bb = G[:, :, FR:, :].reshape(KT, 128, PAIRS, 512)
    W4s = np.empty((PAIRS, 128, KT, 1024), dtype=BF16_NP)
    W4s[..., :512] = A.transpose(2, 1, 0, 3).astype(BF16_NP)
    W4s[..., 512:] = Bb.transpose(2, 1, 0, 3).astype(BF16_NP)

    gb = b4.reshape(H, W)[rows]                            # [40, 256]
    b4s = np.empty((PAIRS, 1, 1024), dtype=BF16_NP)
    b4s[:, 0, :512] = gb[:FR].reshape(PAIRS, 512).astype(BF16_NP)
    b4s[:, 0, 512:] = gb[FR:].reshape(PAIRS, 512).astype(BF16_NP)

    y0 = c * OWN
    Pr = P.reshape(B, H, W)
    Ps = np.concatenate(
        [
            Pr[:, y0 : y0 + HR, :].reshape(B, HR * W),
            Pr[:, y0 + HR : y0 + OWN, :].reshape(B, HR * W),
        ],
        axis=0,
    ).astype(np.float32)
    return {"W4s": W4s, "b4s": b4s, "Ps": np.ascontiguousarray(Ps)}


def make_in_maps(inputs):
    shared = _prep_shared(inputs)
    W4 = np.asarray(inputs["W4"], dtype=np.float32)
    b4 = np.asarray(inputs["b4"], dtype=np.float32)
    P = np.asarray(inputs["P"], dtype=np.float32)
    in_maps = []
    for c in range(NCORES):
        m = dict(shared)
        m.update(_prep_core(c, W4, b4, P))
        in_maps.append(m)
    return in_maps


def assemble_output(results):
    outf = np.empty((B, H, W), dtype=np.float32)
    for c in range(NCORES):
        oc = np.asarray(results[c]["out"])
        y0 = c * OWN
        outf[:, y0 : y0 + HR, :] = oc[:64].reshape(B, HR, W)
        outf[:, y0 + HR : y0 + OWN, :] = oc[64:].reshape(B, HR, W)
    return outf.reshape(B, H * W)


def get_program():
    if "nc" not in _PROGRAM_CACHE:
        _PROGRAM_CACHE["nc"] = _build_program()
    return _PROGRAM_CACHE["nc"]


def kernel(**inputs):
    nc = get_program()
    in_maps = make_in_maps(inputs)
    res = run_bass_kernel_spmd(nc, in_maps, list(range(NCORES)))
    return assemble_output(res.results)
